# revision 35
# baseline (speedup 1.0000x reference)
"""Trainium2 Bass kernel for nn_DMGAGRUcell (GRU cell with graph-conv gates).

Math (per batch b):
  x    = [inputs | hx]                      (N, 66)
  x1   = S @ x, x2 = adp[b] @ x             (diffusion + adaptive hop)
  ru   = sigmoid([x|x1|x2]_interleaved @ W_ru);  r, u = split(ru)
  c    = tanh([x|x1|x2']_interleaved @ W_c)  with x' = [inputs | r*hx]
  out  = u*hx + (1-u)*c
Sharding: 2 batches per core x 8 cores (data parallel over B=16).

Device strategy (145.7us baseline -> 63.5us):
- The four big N x N streams per batch (S@hx, adp@hx, S@rh, adp@rh) run as
  fp8e4m3 DoubleRow matmuls: lhsT [128, 2, 64] / rhs [128, 2, 256] packs two
  128-node k-chunks per instruction at 0.5 cycles/row -> 4x bf16 stream
  throughput (the PE cost model charges out-free-size x cycles; Ldweights
  is free).  fp8 scales (S*16, adp*2048) are folded into the gate weights.
- The 2-wide inp-feature parts (S@inp, adp@inp) are computed exactly on the
  host and shipped as 2-row tiles; feature rows are reordered to [hx | inp]
  so every drain lands at partition base 0.
- S/adp arrive as 512-column stripes carrying all 16 k-chunks, so each
  stream pair completes per-stripe and gconv1 + the ru gate pipeline behind
  the DMA.  adp[1] is sent last: batch 0's gconv2 overlaps its arrival, and
  only batch 1's short chain trails the final stripe.
- Each 512-col stream slab accumulates both 256-col DoubleRow groups in one
  2KB psum bank (single zero region) and drains with one 512-wide copy,
  alternating DVE/ACT.  All activations are pinned to one ACT function set
  (sigmoid_and_others) to avoid 1.3us LoadActFuncSet switches.
- rh transposes (PE + ident) are pipelined per ru slab; u*hx and (1-u) are
  precomputed on GpSimd so the final combine is two bf16 ops.
"""

import numpy as np
import ml_dtypes

BF16 = ml_dtypes.bfloat16
FP8 = ml_dtypes.float8_e4m3fn

N = 2048
B = 16
D_IN = 2
UNITS = 64
F = 66
B_LOC = 2          # batches per core
N_CORES = 8
KC = 16            # k chunks of 128 nodes
JP = 8             # k-chunk pairs (DoubleRow)
NG = 8             # 256-col output groups per stream
NS = 4             # 512-wide gate slabs

_CACHE = {}


def _build():
    if "nc" in _CACHE:
        return _CACHE["nc"]

    from contextlib import ExitStack
    import concourse.mybir as mybir
    import concourse.tile as tile
    from concourse import bacc

    f32 = mybir.dt.float32
    bf = mybir.dt.bfloat16
    f8 = mybir.dt.float8e4
    AF = mybir.ActivationFunctionType
    DR = mybir.MatmulPerfMode.DoubleRow

    # Force every activation onto the one table that holds copy+sigmoid+tanh
    # (sigmoid_and_others); otherwise the table-load pass flip-flops between
    # function sets and each switch costs a ~1.3us LoadActFuncSet.
    _orig_tables = bacc.get_activation_tables

    def _pinned_tables(arch):
        tabs = _orig_tables(arch)
        keep = {"sigmoid_and_others"}
        pin = {mybir.ActivationFunctionType.Copy,
               mybir.ActivationFunctionType.Identity,
               mybir.ActivationFunctionType.Sigmoid,
               mybir.ActivationFunctionType.Tanh}
        return {name: (fns if name in keep else fns - pin)
                for name, fns in tabs.items()}

    bacc.get_activation_tables = _pinned_tables

    nc = bacc.Bacc("TRN2", target_bir_lowering=False, debug=False,
                   num_devices=N_CORES)

    adp8_d = nc.dram_tensor("adp8", [B_LOC, 128, KC, N], f8, kind="ExternalInput")
    s8_d = nc.dram_tensor("s8", [1, 128, KC, N], f8, kind="ExternalInput")
    xh8_d = nc.dram_tensor("xh8", [128, B_LOC, KC, UNITS], f8, kind="ExternalInput")
    gate0_d = nc.dram_tensor("gate0", [F, B_LOC, N], bf, kind="ExternalInput")
    grows_d = nc.dram_tensor("grows", [D_IN, B_LOC, 3, N], bf, kind="ExternalInput")
    wcat_d = nc.dram_tensor("wcat", [F, 640], bf, kind="ExternalInput")
    out_d = nc.dram_tensor("outT", [B_LOC, UNITS, N], bf, kind="ExternalOutput")

    with tile.TileContext(nc) as tc, ExitStack() as ctx:
        cpool = ctx.enter_context(tc.tile_pool(name="cpool", bufs=1))
        rpool = ctx.enter_context(tc.tile_pool(name="rpool", bufs=2))
        tpool = ctx.enter_context(tc.tile_pool(name="tpool", bufs=2))
        pp_s = ctx.enter_context(tc.tile_pool(name="pp_s", bufs=4, space="PSUM"))
        pp_g = ctx.enter_context(tc.tile_pool(name="pp_g", bufs=2, space="PSUM"))
        pp_t = ctx.enter_context(tc.tile_pool(name="pp_t", bufs=2, space="PSUM"))

        # ---- SBUF tiles ----
        s8_t = cpool.tile([128, KC, N], f8, tag="s8", name="s8_t")
        adp_t = [cpool.tile([128, KC, N], f8, tag=f"adp{b}", name=f"adp_t{b}")
                 for b in range(B_LOC)]
        xh8cat = cpool.tile([128, B_LOC, KC, UNITS], f8, tag="xh8",
                            name="xh8cat")
        xh8 = [xh8cat[:, b] for b in range(B_LOC)]
        gate_in = cpool.tile([F, B_LOC, 4, N], bf, tag="gate_in",
                             name="gate_in")
        x0T = [gate_in[:, b, 0] for b in range(B_LOC)]
        x1T = [gate_in[:, b, 1] for b in range(B_LOC)]
        x2T = [gate_in[:, b, 2] for b in range(B_LOC)]
        x0p = [gate_in[:, b, 3] for b in range(B_LOC)]
        rh8nm = [cpool.tile([128, KC, UNITS], f8, tag=f"rh8nm_{b}",
                            name=f"rh8nm_{b}") for b in range(B_LOC)]
        uact = [cpool.tile([UNITS, N], bf, tag=f"uact_{b}", name=f"uact_{b}")
                for b in range(B_LOC)]
        uh = [cpool.tile([UNITS, N], bf, tag=f"uh_{b}", name=f"uh_{b}")
              for b in range(B_LOC)]
        wg = [cpool.tile([UNITS, N], bf, tag=f"wg_{b}", name=f"wg_{b}")
              for b in range(B_LOC)]
        cT = [cpool.tile([UNITS, N], bf, tag=f"cT_{b}", name=f"cT_{b}")
              for b in range(B_LOC)]
        outT = [cpool.tile([UNITS, N], bf, tag=f"outT_{b}", name=f"outT_{b}")
                for b in range(B_LOC)]
        wcat = cpool.tile([F, 640], bf, tag="wcat", name="wcat")
        wru = wcat[:, 0:384]
        wc = wcat[:, 384:576]
        id8 = wcat[0:UNITS, 576:640]

        # ---- DMA order = arrival order (SP queue is in-order). ----
        # lhsT first, then S / adp[0] in 512-col stripes (all 16 k-chunks per
        # stripe -> full contraction available per stripe), with the small
        # gate inputs slotted between stripes, then adp[1].
        nc.sync.dma_start(xh8cat[:], xh8_d[:])
        for s in range(NS):
            sl = slice(s * 512, (s + 1) * 512)
            nc.sync.dma_start(s8_t[:, :, sl], s8_d[0][:, :, sl])
            nc.sync.dma_start(adp_t[0][:, :, sl], adp8_d[0][:, :, sl])
            if s == 0:
                nc.sync.dma_start(gate_in[:, 0, 0, :], gate0_d[:, 0, :])
                nc.sync.dma_start(gate_in[UNITS:F, :, 1:4, :], grows_d[:])
                nc.sync.dma_start(wcat[:], wcat_d[:])
        nc.sync.dma_start(gate_in[:, 1, 0, :], gate0_d[:, 1, :])
        for s in range(NS):
            sl = slice(s * 512, (s + 1) * 512)
            nc.sync.dma_start(adp_t[1][:, :, sl], adp8_d[1][:, :, sl])

        # warm the ACT function table off the critical path
        dum = cpool.tile([1, 2], f32, tag="dum", name="dum")
        nc.scalar.activation(dum[0:1, 0:1], id8[0:1, 0:1], AF.Sigmoid)
        nc.scalar.activation(dum[0:1, 1:2], id8[0:1, 0:1], AF.Tanh)

        def dr_pair(lhsT, rhs, dst, s, pfx, eng):
            # one 512-col slab of a full N x N fp8 DoubleRow stream: two
            # 256-col halves share one psum bank (one accumulation zone),
            # drained by a single 512-wide copy on DVE or ACT.
            ps = pp_s.tile([UNITS, 512], f32, tag="ps_s", name=f"ps_{pfx}_{s}")
            for h in range(2):
                c0 = s * 512 + h * 256
                for j in range(JP):
                    nc.tensor.matmul(
                        ps[:, h * 256:h * 256 + 256],
                        lhsT[:, 2 * j:2 * j + 2, :],
                        rhs[:, 2 * j:2 * j + 2, c0:c0 + 256],
                        start=(h == 0 and j == 0),
                        stop=(h == 1 and j == JP - 1), perf_mode=DR)
            dsl = slice(s * 512, (s + 1) * 512)
            if eng == "act":
                nc.scalar.activation(dst[0:UNITS, dsl], ps[:], AF.Copy)
            else:
                nc.vector.tensor_copy(dst[0:UNITS, dsl], ps[:])

        def ru_slab(b, s):
            sl = slice(s * 512, (s + 1) * 512)
            ps = pp_g.tile([128, 512], f32, tag="ps_g", name=f"ps_ru{b}_{s}")
            nc.tensor.matmul(ps[:], wru[:, 0:128], x0T[b][:, sl],
                             start=True, stop=False)
            nc.tensor.matmul(ps[:], wru[:, 128:256], x1T[b][:, sl],
                             start=False, stop=False)
            nc.tensor.matmul(ps[:], wru[:, 256:384], x2T[b][:, sl],
                             start=False, stop=True)
            rt = rpool.tile([UNITS, 512], f32, tag="rt", name=f"rt{b}_{s}")
            nc.scalar.activation(rt[:], ps[0:UNITS, :], AF.Sigmoid)
            # rh = r*hx -> bf16 (rows 0-63 of x0p)
            nc.vector.tensor_mul(x0p[b][0:UNITS, sl], rt[:], x0T[b][0:UNITS, sl])
            transposes(b, s)
            nc.scalar.activation(uact[b][:, sl], ps[UNITS:128, :], AF.Sigmoid)
            # precompute u*hx and (1-u) on GpSimd (idle mid-kernel) so the
            # final combine is only two ops on the critical tail
            nc.gpsimd.tensor_mul(uh[b][:, sl], uact[b][:, sl],
                                 x0T[b][0:UNITS, sl])
            nc.gpsimd.tensor_scalar(wg[b][:, sl], uact[b][:, sl], -1.0, 1.0,
                                    mybir.AluOpType.mult, mybir.AluOpType.add)

        def transposes(b, s=None):
            blks = range(KC) if s is None else range(4 * s, 4 * s + 4)
            for blk in blks:
                pt = pp_t.tile([128, 1024], bf, tag="ps_t", name=f"pt{b}_{blk}")
                nc.tensor.transpose(
                    pt[:, 0:UNITS],
                    x0p[b][0:UNITS, blk * 128:(blk + 1) * 128], id8[:])
                if blk % 2 == 0:
                    nc.vector.tensor_copy(rh8nm[b][:, blk, :], pt[:, 0:UNITS])
                else:
                    nc.scalar.activation(rh8nm[b][:, blk, :], pt[:, 0:UNITS],
                                         AF.Copy)

        def c_slab(b, s):
            sl = slice(s * 512, (s + 1) * 512)
            psf = pp_g.tile([128, 512], f32, tag="ps_g", name=f"ps_c{b}_{s}")
            ps = psf[0:UNITS, :]
            nc.tensor.matmul(ps, wc[:, 0:64], x0p[b][:, sl],
                             start=True, stop=False)
            nc.tensor.matmul(ps, wc[:, 64:128], x1T[b][:, sl],
                             start=False, stop=False)
            nc.tensor.matmul(ps, wc[:, 128:192], x2T[b][:, sl],
                             start=False, stop=True)
            nc.scalar.activation(cT[b][:, sl], ps, AF.Tanh)
            # out = (1-u)*c + u*hx, two bf16 ops on the tail
            eng = nc.vector if (b == 1 or s % 2 == 1) else nc.gpsimd
            eng.tensor_mul(outT[b][:, sl], wg[b][:, sl], cT[b][:, sl])
            eng.tensor_add(outT[b][:, sl], outT[b][:, sl], uh[b][:, sl])

        # ---- gconv1, pipelined behind the S/adp[0] stripe DMAs.  ru
        # slabs trail one stripe behind so their matmuls never queue ahead
        # of stream pairs on the in-order PE (each ru stripe's drains are
        # long done when its matmuls issue).
        # Batch 0 gconv1 pipelines behind the S/adp0 stripes; ru slabs
        # trail one stripe so their matmuls never block stream pairs on the
        # in-order PE queue (each slab's drains are done when it issues).
        for s in range(NS):
            dr_pair(xh8[0], s8_t, x1T[0], s, "s1b0", "dve")
            dr_pair(xh8[1], s8_t, x1T[1], s, "s1b1", "act")
            dr_pair(xh8[0], adp_t[0], x2T[0], s, "a1b0", "dve")
            if s >= 1:
                ru_slab(0, s - 1)
        ru_slab(0, NS - 1)

        # ---- batch 0 gconv2 interleaved with batch 1 gconv1-adp, whose
        # stripes are still arriving (one pair slotted per ~3us stripe) ----
        dr_pair(rh8nm[0], s8_t, x1T[0], 0, "s2b0", "dve")
        dr_pair(xh8[1], adp_t[1], x2T[1], 0, "a1b1", "dve")
        for s in range(1, NS):
            dr_pair(rh8nm[0], s8_t, x1T[0], s, "s2b0",
                    "act" if s % 2 else "dve")
        dr_pair(rh8nm[0], adp_t[0], x2T[0], 0, "a2b0", "act")
        dr_pair(rh8nm[0], adp_t[0], x2T[0], 1, "a2b0", "dve")
        dr_pair(xh8[1], adp_t[1], x2T[1], 1, "a1b1", "dve")
        ru_slab(1, 0)
        dr_pair(rh8nm[0], adp_t[0], x2T[0], 2, "a2b0", "act")
        dr_pair(rh8nm[0], adp_t[0], x2T[0], 3, "a2b0", "dve")
        dr_pair(xh8[1], adp_t[1], x2T[1], 2, "a1b1", "dve")
        ru_slab(1, 1)
        c_slab(0, 0)
        c_slab(0, 1)
        dr_pair(xh8[1], adp_t[1], x2T[1], 3, "a1b1", "dve")
        ru_slab(1, 2)
        ru_slab(1, 3)

        # ---- batch 1 gconv2 + c; batch 0's remaining c slabs fill the
        # drain-latency bubbles between stream pairs ----
        dr_pair(rh8nm[1], s8_t, x1T[1], 0, "s2b1", "dve")
        c_slab(0, 2)
        dr_pair(rh8nm[1], s8_t, x1T[1], 1, "s2b1", "dve")
        c_slab(0, 3)
        dr_pair(rh8nm[1], s8_t, x1T[1], 2, "s2b1", "dve")
        dr_pair(rh8nm[1], s8_t, x1T[1], 3, "s2b1", "dve")
        for s in range(NS):
            dr_pair(rh8nm[1], adp_t[1], x2T[1], s, "a2b1", "dve")
            if s == NS - 1:
                c_slab(1, 0)
        for s in range(1, NS):
            c_slab(1, s)

        nc.sync.dma_start(out_d[0], outT[0][:])
        for s in range(NS):
            sl = slice(s * 512, (s + 1) * 512)
            nc.sync.dma_start(out_d[1][:, sl], outT[1][:, sl])

    nc.compile()
    _CACHE["nc"] = nc
    return nc


def _prep_host(inputs, hx, adp, support_rows, support_cols, support_vals,
               W_ru, W_c):
    inp = np.ascontiguousarray(inputs.reshape(B, N, D_IN), np.float32)
    hxm = np.ascontiguousarray(hx.reshape(B, N, UNITS), np.float32)

    S = np.zeros((N, N), np.float32)
    np.add.at(S, (support_rows, support_cols), support_vals)

    # fp8 stream operands; scales (x16 / x2048) are folded into the weights
    s8 = (S.T * 16.0).astype(FP8).reshape(KC, 128, N).transpose(1, 0, 2)
    s8 = np.ascontiguousarray(s8)[None]
    adp8 = np.empty((B, 128, KC, N), FP8)
    for b in range(B):
        adp8[b] = (adp[b].T * 2048.0).astype(FP8).reshape(
            KC, 128, N).transpose(1, 0, 2)
    xh8 = np.ascontiguousarray(
        hxm.reshape(B, KC, 128, UNITS).transpose(0, 2, 1, 3)).astype(FP8)

    # feature-major gate inputs, rows reordered to [hx | inp]
    x0T = np.concatenate(
        [hxm.transpose(0, 2, 1), inp.transpose(0, 2, 1)], axis=1).astype(BF16)
    # exact inp-feature parts of the streams (reused by both gconvs)
    si = (16.0 * np.einsum('nm,bmf->bfn', S, inp)).astype(BF16)
    ai = (2048.0 * np.einsum('bnm,bmf->bfn', adp, inp)).astype(BF16)

    def reord(w):  # rows [inp(2) | hx(64)] -> [hx | inp]
        return np.concatenate([w[D_IN:F], w[0:D_IN]], axis=0)

    wcat = np.zeros((F, 640), np.float32)
    wcat[:, 0:128] = reord(W_ru[0::3])
    wcat[:, 128:256] = reord(W_ru[1::3]) / 16.0
    wcat[:, 256:384] = reord(W_ru[2::3]) / 2048.0
    wcat[:, 384:448] = reord(W_c[0::3])
    wcat[:, 448:512] = reord(W_c[1::3]) / 16.0
    wcat[:, 512:576] = reord(W_c[2::3]) / 2048.0
    wcat[0:UNITS, 576:640] = np.eye(UNITS)
    wcat = wcat.astype(BF16)

    shared = {"s8": s8, "wcat": wcat}
    in_maps = []
    for c in range(N_CORES):
        lo, hi = c * B_LOC, (c + 1) * B_LOC
        grows = np.empty((D_IN, B_LOC, 3, N), BF16)
        for b in range(B_LOC):
            grows[:, b, 0] = si[lo + b]
            grows[:, b, 1] = ai[lo + b]
            grows[:, b, 2] = x0T[lo + b][UNITS:F]
        in_maps.append({
            "adp8": np.ascontiguousarray(adp8[lo:hi]),
            "xh8": np.ascontiguousarray(xh8[lo:hi].transpose(1, 0, 2, 3)),
            "gate0": np.ascontiguousarray(x0T[lo:hi].transpose(1, 0, 2)),
            "grows": grows,
            "wcat": wcat,
            "s8": s8,
        })
    return in_maps


def kernel(inputs, hx, adp, support_rows, support_cols, support_vals,
           W_ru, W_c, time_axis=None):
    from concourse.bass_utils import run_bass_kernel_spmd

    inputs = np.asarray(inputs, dtype=np.float32)
    hx = np.asarray(hx, dtype=np.float32)
    adp = np.asarray(adp, dtype=np.float32)
    support_rows = np.asarray(support_rows)
    support_cols = np.asarray(support_cols)
    support_vals = np.asarray(support_vals, dtype=np.float32)
    W_ru = np.asarray(W_ru, dtype=np.float32)
    W_c = np.asarray(W_c, dtype=np.float32)

    nc = _build()
    in_maps = _prep_host(inputs, hx, adp, support_rows, support_cols,
                         support_vals, W_ru, W_c)

    res = run_bass_kernel_spmd(nc, in_maps, core_ids=list(range(N_CORES)),
                               trace=False)
    _CACHE["last_result"] = res

    out = np.empty((B, N * UNITS), np.float32)
    for c in range(N_CORES):
        outT = res.results[c]["outT"]  # (B_LOC, 64, N) bf16
        for i in range(B_LOC):
            out[c * B_LOC + i] = np.ascontiguousarray(
                outT[i].astype(np.float32).T).reshape(N * UNITS)
    return out


# revision 36
# speedup vs baseline: 1.0082x; 1.0082x over previous
"""Trainium2 Bass kernel for nn_DMGAGRUcell (GRU cell with graph-conv gates).

Math (per batch b):
  x    = [inputs | hx]                      (N, 66)
  x1   = S @ x, x2 = adp[b] @ x             (diffusion + adaptive hop)
  ru   = sigmoid([x|x1|x2]_interleaved @ W_ru);  r, u = split(ru)
  c    = tanh([x|x1|x2']_interleaved @ W_c)  with x' = [inputs | r*hx]
  out  = u*hx + (1-u)*c

Sharding: 2 batches per core x 8 cores (data parallel over B=16).

Device strategy: the four big N x N streams per batch (S@hx, adp@hx, S@rh,
adp@rh) run as fp8e4m3 DoubleRow matmuls (two 128-node k-chunks per
instruction, 0.5 cycles/row -> 4x bf16 stream throughput). The tiny
inp-feature parts (S@inp, adp@inp, 2 cols) are computed exactly on the host
and shipped as 2-row tiles. fp8 scale factors (S*16, adp*2048) are folded
into the gate weights host-side. Feature rows are reordered to [hx | inp]
so every drain lands at partition base 0. S/adp arrive as 512-column
stripes (all 16 k-chunks per stripe) so gconv1 and the ru gate pipeline
behind the DMA; gates run bf16.
"""

import numpy as np
import ml_dtypes

BF16 = ml_dtypes.bfloat16
FP8 = ml_dtypes.float8_e4m3fn

N = 2048
B = 16
D_IN = 2
UNITS = 64
F = 66
B_LOC = 2          # batches per core
N_CORES = 8
KC = 16            # k chunks of 128 nodes
JP = 8             # k-chunk pairs (DoubleRow)
NG = 8             # 256-col output groups per stream
NS = 4             # 512-wide gate slabs

_CACHE = {}


def _build():
    if "nc" in _CACHE:
        return _CACHE["nc"]

    from contextlib import ExitStack
    import concourse.mybir as mybir
    import concourse.tile as tile
    from concourse import bacc

    f32 = mybir.dt.float32
    bf = mybir.dt.bfloat16
    f8 = mybir.dt.float8e4
    AF = mybir.ActivationFunctionType
    DR = mybir.MatmulPerfMode.DoubleRow

    # Force every activation onto the one table that holds copy+sigmoid+tanh
    # (sigmoid_and_others); otherwise the table-load pass flip-flops between
    # function sets and each switch costs a ~1.3us LoadActFuncSet.
    _orig_tables = bacc.get_activation_tables

    def _pinned_tables(arch):
        tabs = _orig_tables(arch)
        keep = {"sigmoid_and_others"}
        pin = {mybir.ActivationFunctionType.Copy,
               mybir.ActivationFunctionType.Identity,
               mybir.ActivationFunctionType.Sigmoid,
               mybir.ActivationFunctionType.Tanh}
        return {name: (fns if name in keep else fns - pin)
                for name, fns in tabs.items()}

    bacc.get_activation_tables = _pinned_tables

    nc = bacc.Bacc("TRN2", target_bir_lowering=False, debug=False,
                   num_devices=N_CORES)

    adp8_d = nc.dram_tensor("adp8", [B_LOC, 128, KC, N], f8, kind="ExternalInput")
    s8_d = nc.dram_tensor("s8", [1, 128, KC, N], f8, kind="ExternalInput")
    xh8_d = nc.dram_tensor("xh8", [128, B_LOC, KC, UNITS], f8, kind="ExternalInput")
    gate0_d = nc.dram_tensor("gate0", [F, B_LOC, N], bf, kind="ExternalInput")
    grows_d = nc.dram_tensor("grows", [D_IN, B_LOC, 3, N], bf, kind="ExternalInput")
    wcat_d = nc.dram_tensor("wcat", [F, 640], bf, kind="ExternalInput")
    out_d = nc.dram_tensor("outT", [B_LOC, UNITS, N], bf, kind="ExternalOutput")

    with tile.TileContext(nc) as tc, ExitStack() as ctx:
        cpool = ctx.enter_context(tc.tile_pool(name="cpool", bufs=1))
        rpool = ctx.enter_context(tc.tile_pool(name="rpool", bufs=2))
        tpool = ctx.enter_context(tc.tile_pool(name="tpool", bufs=2))
        pp_s = ctx.enter_context(tc.tile_pool(name="pp_s", bufs=4, space="PSUM"))
        pp_g = ctx.enter_context(tc.tile_pool(name="pp_g", bufs=2, space="PSUM"))
        pp_t = ctx.enter_context(tc.tile_pool(name="pp_t", bufs=2, space="PSUM"))

        # ---- SBUF tiles ----
        s8_t = cpool.tile([128, KC, N], f8, tag="s8", name="s8_t")
        adp_t = [cpool.tile([128, KC, N], f8, tag=f"adp{b}", name=f"adp_t{b}")
                 for b in range(B_LOC)]
        xh8cat = cpool.tile([128, B_LOC, KC, UNITS], f8, tag="xh8",
                            name="xh8cat")
        xh8 = [xh8cat[:, b] for b in range(B_LOC)]
        gate_in = cpool.tile([F, B_LOC, 4, N], bf, tag="gate_in",
                             name="gate_in")
        x0T = [gate_in[:, b, 0] for b in range(B_LOC)]
        x1T = [gate_in[:, b, 1] for b in range(B_LOC)]
        x2T = [gate_in[:, b, 2] for b in range(B_LOC)]
        x0p = [gate_in[:, b, 3] for b in range(B_LOC)]
        rh8nm = [cpool.tile([128, KC, UNITS], f8, tag=f"rh8nm_{b}",
                            name=f"rh8nm_{b}") for b in range(B_LOC)]
        uact = [cpool.tile([UNITS, N], bf, tag=f"uact_{b}", name=f"uact_{b}")
                for b in range(B_LOC)]
        uh = [cpool.tile([UNITS, N], bf, tag=f"uh_{b}", name=f"uh_{b}")
              for b in range(B_LOC)]
        wg = [cpool.tile([UNITS, N], bf, tag=f"wg_{b}", name=f"wg_{b}")
              for b in range(B_LOC)]
        cT = [cpool.tile([UNITS, N], bf, tag=f"cT_{b}", name=f"cT_{b}")
              for b in range(B_LOC)]
        outT = [cpool.tile([UNITS, N], bf, tag=f"outT_{b}", name=f"outT_{b}")
                for b in range(B_LOC)]
        wcat = cpool.tile([F, 640], bf, tag="wcat", name="wcat")
        wru = wcat[:, 0:384]
        wc = wcat[:, 384:576]
        id8 = wcat[0:UNITS, 576:640]

        # ---- DMA order = arrival order (SP queue is in-order). ----
        # lhsT first, then S / adp[0] in 512-col stripes (all 16 k-chunks per
        # stripe -> full contraction available per stripe), with the small
        # gate inputs slotted between stripes, then adp[1].
        nc.sync.dma_start(xh8cat[:], xh8_d[:])
        for s in range(NS):
            sl = slice(s * 512, (s + 1) * 512)
            nc.sync.dma_start(s8_t[:, :, sl], s8_d[0][:, :, sl])
            nc.sync.dma_start(adp_t[0][:, :, sl], adp8_d[0][:, :, sl])
            if s == 0:
                nc.sync.dma_start(gate_in[:, 0, 0, :], gate0_d[:, 0, :])
                nc.sync.dma_start(gate_in[UNITS:F, :, 1:4, :], grows_d[:])
                nc.sync.dma_start(wcat[:], wcat_d[:])
        nc.sync.dma_start(gate_in[:, 1, 0, :], gate0_d[:, 1, :])
        for s in range(NS):
            sl = slice(s * 512, (s + 1) * 512)
            nc.sync.dma_start(adp_t[1][:, :, sl], adp8_d[1][:, :, sl])

        # warm the ACT function table off the critical path
        dum = cpool.tile([1, 2], f32, tag="dum", name="dum")
        nc.scalar.activation(dum[0:1, 0:1], id8[0:1, 0:1], AF.Sigmoid)
        nc.scalar.activation(dum[0:1, 1:2], id8[0:1, 0:1], AF.Tanh)

        def dr_pair(lhsT, rhs, dst, s, pfx, eng):
            # one 512-col slab of a full N x N fp8 DoubleRow stream: two
            # 256-col halves share one psum bank (one accumulation zone),
            # drained by a single 512-wide copy on DVE or ACT.
            ps = pp_s.tile([UNITS, 512], f32, tag="ps_s", name=f"ps_{pfx}_{s}")
            for h in range(2):
                c0 = s * 512 + h * 256
                for j in range(JP):
                    nc.tensor.matmul(
                        ps[:, h * 256:h * 256 + 256],
                        lhsT[:, 2 * j:2 * j + 2, :],
                        rhs[:, 2 * j:2 * j + 2, c0:c0 + 256],
                        start=(h == 0 and j == 0),
                        stop=(h == 1 and j == JP - 1), perf_mode=DR)
            dsl = slice(s * 512, (s + 1) * 512)
            if eng == "act":
                nc.scalar.activation(dst[0:UNITS, dsl], ps[:], AF.Copy)
            else:
                nc.vector.tensor_copy(dst[0:UNITS, dsl], ps[:])

        def ru_slab(b, s):
            sl = slice(s * 512, (s + 1) * 512)
            ps = pp_g.tile([128, 512], f32, tag="ps_g", name=f"ps_ru{b}_{s}")
            nc.tensor.matmul(ps[:], wru[:, 0:128], x0T[b][:, sl],
                             start=True, stop=False)
            nc.tensor.matmul(ps[:], wru[:, 128:256], x1T[b][:, sl],
                             start=False, stop=False)
            nc.tensor.matmul(ps[:], wru[:, 256:384], x2T[b][:, sl],
                             start=False, stop=True)
            rt = rpool.tile([UNITS, 512], f32, tag="rt", name=f"rt{b}_{s}")
            nc.scalar.activation(rt[:], ps[0:UNITS, :], AF.Sigmoid)
            # rh = r*hx -> bf16 (rows 0-63 of x0p)
            nc.vector.tensor_mul(x0p[b][0:UNITS, sl], rt[:], x0T[b][0:UNITS, sl])
            transposes(b, s)
            nc.scalar.activation(uact[b][:, sl], ps[UNITS:128, :], AF.Sigmoid)
            # precompute u*hx and (1-u) on GpSimd (idle mid-kernel) so the
            # final combine is only two ops on the critical tail
            nc.gpsimd.tensor_mul(uh[b][:, sl], uact[b][:, sl],
                                 x0T[b][0:UNITS, sl])
            nc.gpsimd.tensor_scalar(wg[b][:, sl], uact[b][:, sl], -1.0, 1.0,
                                    mybir.AluOpType.mult, mybir.AluOpType.add)

        def transposes(b, s=None):
            blks = range(KC) if s is None else range(4 * s, 4 * s + 4)
            for blk in blks:
                pt = pp_t.tile([128, 1024], bf, tag="ps_t", name=f"pt{b}_{blk}")
                nc.tensor.transpose(
                    pt[:, 0:UNITS],
                    x0p[b][0:UNITS, blk * 128:(blk + 1) * 128], id8[:])
                if blk % 2 == 0:
                    nc.vector.tensor_copy(rh8nm[b][:, blk, :], pt[:, 0:UNITS])
                else:
                    nc.scalar.activation(rh8nm[b][:, blk, :], pt[:, 0:UNITS],
                                         AF.Copy)

        def c_slab(b, s):
            sl = slice(s * 512, (s + 1) * 512)
            psf = pp_g.tile([128, 512], f32, tag="ps_g", name=f"ps_c{b}_{s}")
            ps = psf[0:UNITS, :]
            nc.tensor.matmul(ps, wc[:, 0:64], x0p[b][:, sl],
                             start=True, stop=False)
            nc.tensor.matmul(ps, wc[:, 64:128], x1T[b][:, sl],
                             start=False, stop=False)
            nc.tensor.matmul(ps, wc[:, 128:192], x2T[b][:, sl],
                             start=False, stop=True)
            nc.scalar.activation(cT[b][:, sl], ps, AF.Tanh)
            # out = (1-u)*c + u*hx, two bf16 ops on the tail
            eng = nc.vector if (b == 1 or s % 2 == 1) else nc.gpsimd
            eng.tensor_mul(outT[b][:, sl], wg[b][:, sl], cT[b][:, sl])
            eng.tensor_add(outT[b][:, sl], outT[b][:, sl], uh[b][:, sl])

        # ---- gconv1, pipelined behind the S/adp[0] stripe DMAs.  ru
        # slabs trail one stripe behind so their matmuls never queue ahead
        # of stream pairs on the in-order PE (each ru stripe's drains are
        # long done when its matmuls issue).
        # Batch 0 gconv1 pipelines behind the S/adp0 stripes; ru slabs
        # trail one stripe so their matmuls never block stream pairs on the
        # in-order PE queue (each slab's drains are done when it issues).
        for s in range(NS):
            dr_pair(xh8[0], s8_t, x1T[0], s, "s1b0", "dve")
            dr_pair(xh8[1], s8_t, x1T[1], s, "s1b1", "act")
            dr_pair(xh8[0], adp_t[0], x2T[0], s, "a1b0", "dve")
            if s >= 1:
                ru_slab(0, s - 1)
        ru_slab(0, NS - 1)

        # ---- batch 0 gconv2 interleaved with batch 1 gconv1-adp, whose
        # stripes are still arriving (one pair slotted per ~3us stripe) ----
        for s in range(NS):
            dr_pair(rh8nm[0], s8_t, x1T[0], s, "s2b0",
                    "act" if s % 2 else "dve")
        dr_pair(xh8[1], adp_t[1], x2T[1], 0, "a1b1", "dve")
        dr_pair(rh8nm[0], adp_t[0], x2T[0], 0, "a2b0", "act")
        dr_pair(rh8nm[0], adp_t[0], x2T[0], 1, "a2b0", "act")
        dr_pair(xh8[1], adp_t[1], x2T[1], 1, "a1b1", "dve")
        ru_slab(1, 0)
        dr_pair(rh8nm[0], adp_t[0], x2T[0], 2, "a2b0", "act")
        dr_pair(rh8nm[0], adp_t[0], x2T[0], 3, "a2b0", "act")
        dr_pair(xh8[1], adp_t[1], x2T[1], 2, "a1b1", "dve")
        ru_slab(1, 1)
        c_slab(0, 0)
        c_slab(0, 1)
        dr_pair(xh8[1], adp_t[1], x2T[1], 3, "a1b1", "dve")
        ru_slab(1, 2)
        ru_slab(1, 3)

        # ---- batch 1 gconv2 + c; batch 0's remaining c slabs fill the
        # drain-latency bubbles between stream pairs ----
        dr_pair(rh8nm[1], s8_t, x1T[1], 0, "s2b1", "dve")
        c_slab(0, 2)
        dr_pair(rh8nm[1], s8_t, x1T[1], 1, "s2b1", "dve")
        c_slab(0, 3)
        dr_pair(rh8nm[1], s8_t, x1T[1], 2, "s2b1", "dve")
        dr_pair(rh8nm[1], s8_t, x1T[1], 3, "s2b1", "dve")
        for s in range(NS):
            dr_pair(rh8nm[1], adp_t[1], x2T[1], s, "a2b1", "dve")
            if s == NS - 1:
                c_slab(1, 0)
        for s in range(1, NS):
            c_slab(1, s)

        nc.sync.dma_start(out_d[0], outT[0][:])
        for s in range(NS):
            sl = slice(s * 512, (s + 1) * 512)
            nc.sync.dma_start(out_d[1][:, sl], outT[1][:, sl])

    nc.compile()
    _CACHE["nc"] = nc
    return nc


def _prep_host(inputs, hx, adp, support_rows, support_cols, support_vals,
               W_ru, W_c):
    inp = np.ascontiguousarray(inputs.reshape(B, N, D_IN), np.float32)
    hxm = np.ascontiguousarray(hx.reshape(B, N, UNITS), np.float32)

    S = np.zeros((N, N), np.float32)
    np.add.at(S, (support_rows, support_cols), support_vals)

    # fp8 stream operands; scales (x16 / x2048) are folded into the weights
    s8 = (S.T * 16.0).astype(FP8).reshape(KC, 128, N).transpose(1, 0, 2)
    s8 = np.ascontiguousarray(s8)[None]
    adp8 = np.empty((B, 128, KC, N), FP8)
    for b in range(B):
        adp8[b] = (adp[b].T * 2048.0).astype(FP8).reshape(
            KC, 128, N).transpose(1, 0, 2)
    xh8 = np.ascontiguousarray(
        hxm.reshape(B, KC, 128, UNITS).transpose(0, 2, 1, 3)).astype(FP8)

    # feature-major gate inputs, rows reordered to [hx | inp]
    x0T = np.concatenate(
        [hxm.transpose(0, 2, 1), inp.transpose(0, 2, 1)], axis=1).astype(BF16)
    # exact inp-feature parts of the streams (reused by both gconvs)
    si = (16.0 * np.einsum('nm,bmf->bfn', S, inp)).astype(BF16)
    ai = (2048.0 * np.einsum('bnm,bmf->bfn', adp, inp)).astype(BF16)

    def reord(w):  # rows [inp(2) | hx(64)] -> [hx | inp]
        return np.concatenate([w[D_IN:F], w[0:D_IN]], axis=0)

    wcat = np.zeros((F, 640), np.float32)
    wcat[:, 0:128] = reord(W_ru[0::3])
    wcat[:, 128:256] = reord(W_ru[1::3]) / 16.0
    wcat[:, 256:384] = reord(W_ru[2::3]) / 2048.0
    wcat[:, 384:448] = reord(W_c[0::3])
    wcat[:, 448:512] = reord(W_c[1::3]) / 16.0
    wcat[:, 512:576] = reord(W_c[2::3]) / 2048.0
    wcat[0:UNITS, 576:640] = np.eye(UNITS)
    wcat = wcat.astype(BF16)

    shared = {"s8": s8, "wcat": wcat}
    in_maps = []
    for c in range(N_CORES):
        lo, hi = c * B_LOC, (c + 1) * B_LOC
        grows = np.empty((D_IN, B_LOC, 3, N), BF16)
        for b in range(B_LOC):
            grows[:, b, 0] = si[lo + b]
            grows[:, b, 1] = ai[lo + b]
            grows[:, b, 2] = x0T[lo + b][UNITS:F]
        in_maps.append({
            "adp8": np.ascontiguousarray(adp8[lo:hi]),
            "xh8": np.ascontiguousarray(xh8[lo:hi].transpose(1, 0, 2, 3)),
            "gate0": np.ascontiguousarray(x0T[lo:hi].transpose(1, 0, 2)),
            "grows": grows,
            "wcat": wcat,
            "s8": s8,
        })
    return in_maps


def kernel(inputs, hx, adp, support_rows, support_cols, support_vals,
           W_ru, W_c, time_axis=None):
    from concourse.bass_utils import run_bass_kernel_spmd

    inputs = np.asarray(inputs, dtype=np.float32)
    hx = np.asarray(hx, dtype=np.float32)
    adp = np.asarray(adp, dtype=np.float32)
    support_rows = np.asarray(support_rows)
    support_cols = np.asarray(support_cols)
    support_vals = np.asarray(support_vals, dtype=np.float32)
    W_ru = np.asarray(W_ru, dtype=np.float32)
    W_c = np.asarray(W_c, dtype=np.float32)

    nc = _build()
    in_maps = _prep_host(inputs, hx, adp, support_rows, support_cols,
                         support_vals, W_ru, W_c)

    res = run_bass_kernel_spmd(nc, in_maps, core_ids=list(range(N_CORES)),
                               trace=False)
    _CACHE["last_result"] = res

    out = np.empty((B, N * UNITS), np.float32)
    for c in range(N_CORES):
        outT = res.results[c]["outT"]  # (B_LOC, 64, N) bf16
        for i in range(B_LOC):
            out[c * B_LOC + i] = np.ascontiguousarray(
                outT[i].astype(np.float32).T).reshape(N * UNITS)
    return out
